# revision 58
# baseline (speedup 1.0000x reference)
"""Distributed GQA attention (RoPE, causal) for 8 TRN2 NeuronCores.

Sharding: tensor-parallel over heads (4 Q heads / 1 KV head per core).
Each core computes full-length Q/K/V projections for its heads, RoPE,
flash-style causal attention with the 4 heads packed into N=512 matmuls.
The output projection is sequence-parallel: per batch, attention outputs
([256 feat, 2048 pos] per core) are exchanged with one AllToAll so every
core owns a 256-position block with all 2048 attention features, then
multiplies by the full (resident) wo — no ReduceScatter of [D, RT]
partials and no 16.8MB partial-sum DMA traffic.

Layouts are feature-major ("transposed"): activations live as [feat, row]
so every matmul contracts over the partition dim with base partition 0.
Softmax runs max-free (scores are O(5) here), with the denominator
produced for free by a ones-column appended to V and inverted on the
scalar engine straight out of PSUM.
"""

import numpy as np
import ml_dtypes

B, S, D = 2, 2048, 2048
H, HKV, HD = 32, 8, 64
M = 8                 # cores
HL = H // M           # 4 local Q heads
CH = 128              # position chunk
NCH = S // CH         # 16 chunks per sequence
RT = B * S            # 4096 total rows
QF = HL * HD          # 256 local q features
POS = S // M          # 256 positions owned per core per batch

bf16 = ml_dtypes.bfloat16

_CACHE = {}
RUN_OPTS = {}          # test harness may set {"trace": True}
LAST_RESULT = [None]   # test harness reads profiling info from here


def _build_nc():
    import concourse.bacc as bacc
    import concourse.mybir as mybir
    from concourse import tile

    F32, BF16 = mybir.dt.float32, mybir.dt.bfloat16
    Exp = mybir.ActivationFunctionType.Exp

    nc = bacc.Bacc("TRN2", target_bir_lowering=False, debug=False, num_devices=M)

    xt_p = nc.declare_dram_parameter("xt", [D, RT], BF16, isOutput=False)
    cos_p = nc.declare_dram_parameter("cos2", [128, S], BF16, isOutput=False)
    ssin_p = nc.declare_dram_parameter("ssin2", [128, S], BF16, isOutput=False)
    wq_p = nc.declare_dram_parameter("wqs", [D, QF], BF16, isOutput=False)
    wkv_p = nc.declare_dram_parameter("wkvs", [D, 2 * HD], BF16, isOutput=False)
    wo_p = nc.declare_dram_parameter("woall", [D, D], BF16, isOutput=False)
    tri_p = nc.declare_dram_parameter("tri4", [128, 512], BF16, isOutput=False)
    id_p = nc.declare_dram_parameter("ident", [128, 128], BF16, isOutput=False)
    out_p = nc.declare_dram_parameter("out", [B * POS, D], BF16, isOutput=True)

    with tile.TileContext(nc) as tc:
        with tc.tile_pool(name="dram", bufs=1, space="DRAM") as dram, \
             tc.tile_pool(name="persist", bufs=1) as per, \
             tc.tile_pool(name="xload", bufs=2) as xload, \
             tc.tile_pool(name="work", bufs=2) as work, \
             tc.tile_pool(name="pwork", bufs=2) as pwork, \
             tc.tile_pool(name="ostage", bufs=2) as ostage, \
             tc.tile_pool(name="atpool", bufs=2) as atpool, \
             tc.tile_pool(name="pj", bufs=2, space="PSUM") as pj, \
             tc.tile_pool(name="sc", bufs=2, space="PSUM") as sc, \
             tc.tile_pool(name="acc", bufs=2, space="PSUM") as acc:

            # ---- resident loads -------------------------------------------------
            # rc0/rc1 x slices first: they gate the first two projection rounds
            xr0 = xload.tile([128, 16, 512], BF16, tag="x")
            for _k in range(8):
                nc.sync.dma_start(
                    out=xr0[:, 2 * _k:2 * (_k + 1), :],
                    in_=xt_p[256 * _k:256 * (_k + 1), 0:512].rearrange(
                        "(n p) f -> p n f", p=128))
            wq_sb = per.tile([128, 16, QF], BF16, tag="wq")
            for _h in range(2):
                for _d in range(4):
                    nc.sync.dma_start(
                        out=wq_sb[:, 4 * _d:4 * (_d + 1), 128 * _h:128 * (_h + 1)],
                        in_=wq_p[512 * _d:512 * (_d + 1),
                                 128 * _h:128 * (_h + 1)].rearrange("(n p) f -> p n f", p=128))
            xr1 = xload.tile([128, 16, 512], BF16, tag="x")
            for _k in range(8):
                nc.sync.dma_start(
                    out=xr1[:, 2 * _k:2 * (_k + 1), :],
                    in_=xt_p[256 * _k:256 * (_k + 1), 512:1024].rearrange(
                        "(n p) f -> p n f", p=128))
            wkv_sb = per.tile([128, 16, 2 * HD], BF16, tag="wkv")
            for _d in range(4):
                nc.sync.dma_start(
                    out=wkv_sb[:, 4 * _d:4 * (_d + 1), :],
                    in_=wkv_p[512 * _d:512 * (_d + 1), :].rearrange("(n p) f -> p n f", p=128))
            cos_sb = per.tile([128, S], BF16, tag="cos")
            for _h in range(4):
                nc.sync.dma_start(out=cos_sb[:, 512 * _h:512 * (_h + 1)],
                                  in_=cos_p[:, 512 * _h:512 * (_h + 1)])
            ssin_sb = per.tile([128, S], BF16, tag="ssin")
            for _h in range(4):
                nc.sync.dma_start(out=ssin_sb[:, 512 * _h:512 * (_h + 1)],
                                  in_=ssin_p[:, 512 * _h:512 * (_h + 1)])
            tri_sb = per.tile([128, 512], BF16, tag="tri")
            nc.sync.dma_start(out=tri_sb[:], in_=tri_p[:, :])
            id_sb = per.tile([128, 128], BF16, tag="ident")
            nc.sync.dma_start(out=id_sb[:], in_=id_p[:, :])
            # full wo, pre-transposed: wo_sb[p, d, of] = wo[of, 128*d+p]
            # (loaded late, after rc0's x, to keep startup DMA bandwidth for x)
            wo_sb = per.tile([128, 16, D], BF16, tag="wo")

            # q/kt live on all 128 partitions: rows 64-127 duplicate rows 0-63 so
            # score matmuls run as two concurrent 64-row PE tiles (T0 + T8)
            q_flat = per.tile([128, B * NCH * HL * CH], BF16, tag="qflat")  # (b,c,hh,pos)
            kt_sb = per.tile([128, RT], BF16, tag="kt")
            at_sb = [per.tile([128, RT], BF16, tag=f"at{i}", name=f"at{i}") for i in range(2)]
            vau = [per.tile([128, HD + 1], BF16, tag=f"vau{i}", name=f"vau{i}") for i in range(RT // 128)]
            atall = [None]  # shared SBUF buffer, reloaded per batch

            qv = q_flat[0:64, :].rearrange("p (b c h x) -> p b c h x", b=B, c=NCH, h=HL, x=CH)

            # ---- AllToAll buffers (per half-batch: 4 x 0.5MB) ------------------
            # half (b,h) covers batch-b positions [1024h, 1024h+1024); shard j =
            # my 256 features x 128 positions -> dest core j, which owns batch-b
            # positions 1024h + 128j .. +128.
            a2a_in = [dram.tile([M * QF, CH], BF16, tag=f"a2ai{i}", name=f"a2ai{i}")
                      for i in range(4)]
            a2a_out = [dram.tile([M * QF, CH], BF16, tag=f"a2ao{i}", name=f"a2ao{i}")
                       for i in range(4)]

            def emit_a2a_block(b, cc):
                # chunks cc-1, cc (= shards j0, j0+1 of half h) -> a2a_in rows
                h, j0 = cc // 8, (cc % 8) - 1
                for t in range(2):
                    dst = a2a_in[2 * b + h].rearrange(
                        "(j t p) c -> t p j c", t=2, p=128)[t][:, j0:j0 + 2, :]
                    src = at_sb[t][:, b * S + CH * (cc - 1):b * S + CH * (cc + 1)
                                   ].rearrange("p (j c) -> p j c", j=2)
                    nc.sync.dma_start(out=dst, in_=src)

            def emit_a2a(b, h):
                nc.gpsimd.collective_compute(
                    "AllToAll", mybir.AluOpType.bypass,
                    replica_groups=[list(range(M))],
                    ins=[a2a_in[2 * b + h].opt()], outs=[a2a_out[2 * b + h].opt()])

            def emit_atall_load(b, h):
                # a2a_out rows 128*d+p -> atall[p, d, :]
                at_t = atpool.tile([128, 16, CH], BF16, tag="atall", name="atall")
                atall[0] = at_t
                for k in range(4):
                    nc.sync.dma_start(
                        out=at_t[:, 4 * k:4 * (k + 1), :],
                        in_=a2a_out[2 * b + h][512 * k:512 * (k + 1), :].rearrange(
                            "(d p) c -> p d c", p=128))

            def emit_wo_chain(b, h, o4):
                # out.T chain: lhsT = attention block (stationary), wo streams N=512
                ps = pj.tile([128, 512], F32, tag="pj", name="wops")
                for d in range(16):
                    nc.tensor.matmul(ps[:], atall[0][:, d, :],
                                     wo_sb[:, d, 512 * o4:512 * (o4 + 1)],
                                     start=(d == 0), stop=(d == 15))
                ob = ostage.tile([128, 512], BF16, tag="ob", name="ob")
                nc.any.tensor_copy(ob[:], ps[:])
                nc.sync.dma_start(
                    out=out_p[256 * b + 128 * h:256 * b + 128 * (h + 1),
                              512 * o4:512 * (o4 + 1)],
                    in_=ob[:])

            def emit_attn(b, c):
                o_ps = acc.tile([HD + 1, 512], F32, tag="acc", name="ops")
                qc0 = (b * NCH + c) * 512
                for j0 in range(0, c + 1, 2):
                    js = [j for j in (j0, j0 + 1) if j <= c]
                    s_ps = sc.tile([128, 1024], F32, tag="sc", name="sps")
                    for idx, j in enumerate(js):
                        lo = 64 * idx  # idx 0 -> PE row-tile T0, idx 1 -> T8
                        nc.tensor.matmul(
                            s_ps[:, 512 * idx:512 * (idx + 1)],
                            kt_sb[lo:lo + 64, b * S + CH * j: b * S + CH * (j + 1)],
                            q_flat[lo:lo + 64, qc0:qc0 + 512], start=True, stop=True)
                    nw = 512 * len(js)
                    p_sb = pwork.tile([128, 1024], BF16, tag="p", name="psb")
                    nc.scalar.activation(p_sb[:, 0:nw], s_ps[:, 0:nw], Exp, scale=0.125)
                    if c in js:
                        idx = js.index(c)
                        nc.vector.tensor_mul(p_sb[:, 512 * idx:512 * (idx + 1)],
                                             p_sb[:, 512 * idx:512 * (idx + 1)], tri_sb[:])
                    for idx, j in enumerate(js):
                        nc.tensor.matmul(o_ps[:], vau[b * NCH + j][:],
                                         p_sb[:, 512 * idx:512 * (idx + 1)],
                                         start=(j == 0), stop=(j == c))
                # normalization: 1/den from the PSUM ones-row
                bc = pwork.tile([64, 512], F32, tag="bc", name="bct")
                nc.vector.tensor_copy(bc[0:1, :], o_ps[HD:HD + 1, :])
                rrow = pwork.tile([1, 512], F32, tag="rrow", name="rrow")
                nc.vector.reciprocal_approx_fast(rrow[:], bc[0:1, :])
                nc.gpsimd.partition_broadcast(bc[:], rrow[:])
                for hh in range(HL):
                    nc.vector.tensor_mul(
                        at_sb[hh // 2][64 * (hh % 2):64 * (hh % 2) + 64,
                                       b * S + CH * c: b * S + CH * (c + 1)],
                        o_ps[0:64, 128 * hh:128 * (hh + 1)],
                        bc[:, 128 * hh:128 * (hh + 1)])

            # ---- projections + RoPE, per 512-row slice -------------------------
            pending_blocks = []

            for rc in range(8):
                b, cg = rc // 4, rc % 4
                if rc == 0:
                    xr = xr0
                elif rc == 1:
                    xr = xr1
                else:
                    xr = xload.tile([128, 16, 512], BF16, tag="x")
                    for _k in range(8):
                        nc.sync.dma_start(
                            out=xr[:, 2 * _k:2 * (_k + 1), :],
                            in_=xt_p[256 * _k:256 * (_k + 1),
                                     512 * rc:512 * (rc + 1)].rearrange(
                                         "(n p) f -> p n f", p=128))
                # flush the previous slice's a2a shard writes now — after the x
                # triggers, so their norm-waits can't block the x pipeline
                for (pb, pcc) in pending_blocks:
                    emit_a2a_block(pb, pcc)
                pending_blocks.clear()
                if rc == 2:
                    emit_a2a(0, 0)
                if rc == 1:
                    for _k in range(8):
                        nc.sync.dma_start(
                            out=wo_sb[:, 2 * _k:2 * (_k + 1), :],
                            in_=wo_p[256 * _k:256 * (_k + 1), :].rearrange(
                                "(d p) f -> p d f", p=128))
                if rc == 4:
                    emit_atall_load(0, 0)
                if rc == 6:
                    emit_atall_load(0, 1)
                if rc == 7:
                    emit_atall_load(1, 0)
                cs = cos_sb[:, 512 * cg:512 * (cg + 1)]
                sn = ssin_sb[:, 512 * cg:512 * (cg + 1)]

                # Q: two 128-feature chunks (2 heads each)
                for f in range(2):
                    ps = pj.tile([128, 512], F32, tag="pj")
                    for d in range(16):
                        nc.tensor.matmul(ps[:], wq_sb[:, d, 128 * f:128 * (f + 1)],
                                         xr[:, d, :], start=(d == 0), stop=(d == 15))
                    t1 = work.tile([128, 512], BF16, tag="t1")
                    nc.vector.tensor_mul(t1[:], ps[:], cs)
                    sw = work.tile([128, 512], BF16, tag="sw")
                    for a, bq in ((0, 1), (1, 0), (2, 3), (3, 2)):
                        nc.scalar.copy(sw[32 * a:32 * (a + 1), :], ps[32 * bq:32 * (bq + 1), :])
                    t2 = work.tile([128, 512], BF16, tag="t2")
                    nc.vector.tensor_mul(t2[:], sw[:], sn)
                    for hf in range(2):
                        hh = 2 * f + hf
                        dst = qv[:, b, 4 * cg:4 * (cg + 1), hh, :]
                        nc.vector.tensor_add(
                            dst,
                            t1[64 * hf:64 * (hf + 1), :].rearrange("p (a x) -> p a x", x=CH),
                            t2[64 * hf:64 * (hf + 1), :].rearrange("p (a x) -> p a x", x=CH))
                    qv2 = q_flat[64:128, :].rearrange("p (b c h x) -> p b c h x",
                                                      b=B, c=NCH, h=HL, x=CH)
                    nc.scalar.copy(qv2[:, b, 4 * cg:4 * (cg + 1), 2 * f:2 * f + 2, :],
                                   qv[:, b, 4 * cg:4 * (cg + 1), 2 * f:2 * f + 2, :])

                # K+V packed: one full-array matmul chain (k rows 0-63, v rows 64-127)
                ps = pj.tile([128, 512], F32, tag="pj")
                for d in range(16):
                    nc.tensor.matmul(ps[:], wkv_sb[:, d, :], xr[:, d, :],
                                     start=(d == 0), stop=(d == 15))
                t1 = work.tile([128, 512], BF16, tag="t1")
                nc.vector.tensor_mul(t1[0:64, :], ps[0:64, :], cs[0:64, :])
                sw = work.tile([128, 512], BF16, tag="sw")
                nc.scalar.copy(sw[0:32, :], ps[32:64, :])
                nc.scalar.copy(sw[32:64, :], ps[0:32, :])
                t2 = work.tile([128, 512], BF16, tag="t2")
                nc.vector.tensor_mul(t2[0:64, :], sw[0:64, :], sn[0:64, :])
                nc.vector.tensor_add(kt_sb[0:64, 512 * rc:512 * (rc + 1)], t1[0:64, :], t2[0:64, :])
                nc.scalar.copy(kt_sb[64:128, 512 * rc:512 * (rc + 1)],
                               kt_sb[0:64, 512 * rc:512 * (rc + 1)])

                vt = work.tile([128, 512], BF16, tag="sw")
                nc.vector.tensor_copy(vt[0:64, :], ps[64:128, :])
                for t in range(4):
                    tp = acc.tile([128, 64], F32, tag="acc")
                    nc.tensor.matmul(tp[:], vt[0:64, 128 * t:128 * (t + 1)], id_sb[0:64, 0:64],
                                     start=True, stop=True)
                    vtile = vau[4 * rc + t]
                    nc.vector.tensor_copy(vtile[:, 0:HD], tp[:])
                    nc.vector.memset(vtile[:, HD:HD + 1], 1.0)

                for cc in range(4 * cg, 4 * cg + 4):
                    emit_attn(b, cc)
                    # queue finished 2-chunk shard pairs for the A2A inputs
                    if cc % 2 == 1:
                        pending_blocks.append((b, cc))
                    # trigger each half's A2A once its shards are written (the
                    # cross-batch ones a little late so the gpsimd queue isn't
                    # head-of-line blocked on the a2a_in DMA waits)
                    if b == 1 and cc == 1:
                        emit_a2a(0, 1)
                    if b == 1 and cc == 9:
                        emit_a2a(1, 0)
                    # interleave wo chains of already-exchanged halves
                    if b == 1 and 4 <= cc < 8:
                        emit_wo_chain(0, 0, cc - 4)
                    if b == 1 and 8 <= cc < 12:
                        emit_wo_chain(0, 1, cc - 8)

            for (pb, pcc) in pending_blocks:
                emit_a2a_block(pb, pcc)
            pending_blocks.clear()
            emit_a2a(1, 1)
            # wo chains for b1h0 fill the PE during the final AllToAll
            for o4 in range(4):
                emit_wo_chain(1, 0, o4)
            emit_atall_load(1, 1)
            for o4 in range(4):
                emit_wo_chain(1, 1, o4)

    nc.compile()
    return nc


def _stage(x, cos, sin, wq, wk, wv, wo):
    xt = np.ascontiguousarray(x.reshape(RT, D).T).astype(bf16)
    cosT = cos.T.astype(np.float32)                      # [64, S]
    sinT = sin.T.astype(np.float32)
    cos2 = np.concatenate([cosT, cosT], axis=0).astype(bf16)       # [128, S]
    ssin1 = np.concatenate([-sinT[:HD // 2], sinT[HD // 2:]], axis=0)
    ssin2 = np.concatenate([ssin1, ssin1], axis=0).astype(bf16)    # [128, S]
    tri4 = np.tile(np.triu(np.ones((CH, CH), np.float32)), (1, 4)).astype(bf16)
    ident = np.eye(128, dtype=np.float32).astype(bf16)
    woall = np.ascontiguousarray(wo.T).astype(bf16)      # [af, of]

    in_maps = []
    for m in range(M):
        in_maps.append({
            "xt": xt,
            "cos2": cos2,
            "ssin2": ssin2,
            "wqs": np.ascontiguousarray(wq[QF * m:QF * (m + 1), :].T).astype(bf16),
            "wkvs": np.ascontiguousarray(np.concatenate(
                [wk[HD * m:HD * (m + 1), :].T, wv[HD * m:HD * (m + 1), :].T], axis=1)).astype(bf16),
            "woall": woall,
            "tri4": tri4,
            "ident": ident,
        })
    return in_maps


def kernel(x, cos, sin, wq, wk, wv, wo):
    from concourse.bass_utils import run_bass_kernel_spmd

    if "nc" not in _CACHE:
        _CACHE["nc"] = _build_nc()
    nc = _CACHE["nc"]

    in_maps = _stage(x, cos, sin, wq, wk, wv, wo)
    res = run_bass_kernel_spmd(nc, in_maps, list(range(M)), **RUN_OPTS)
    LAST_RESULT[0] = res

    full = np.empty((B, S, D), np.float32)
    for m in range(M):
        o = np.asarray(res.results[m]["out"]).astype(np.float32)   # [4*CH, D]
        for b in range(B):
            for h in range(2):
                full[b, 1024 * h + CH * m:1024 * h + CH * (m + 1), :] = \
                    o[256 * b + CH * h:256 * b + CH * (h + 1), :]
    return full


# revision 60
# speedup vs baseline: 1.0276x; 1.0276x over previous
"""Distributed GQA attention (RoPE, causal) for 8 TRN2 NeuronCores.

Sharding: tensor-parallel over heads (4 Q heads / 1 KV head per core).
Each core computes full-length Q/K/V projections for its heads, RoPE,
flash-style causal attention with the 4 heads packed into N=512 matmuls.
The output projection is sequence-parallel: per batch, attention outputs
([256 feat, 2048 pos] per core) are exchanged with one AllToAll so every
core owns a 256-position block with all 2048 attention features, then
multiplies by the full (resident) wo — no ReduceScatter of [D, RT]
partials and no 16.8MB partial-sum DMA traffic.

Layouts are feature-major ("transposed"): activations live as [feat, row]
so every matmul contracts over the partition dim with base partition 0.
Softmax runs max-free (scores are O(5) here), with the denominator
produced for free by a ones-column appended to V and inverted on the
scalar engine straight out of PSUM.
"""

import numpy as np
import ml_dtypes

B, S, D = 2, 2048, 2048
H, HKV, HD = 32, 8, 64
M = 8                 # cores
HL = H // M           # 4 local Q heads
CH = 128              # position chunk
NCH = S // CH         # 16 chunks per sequence
RT = B * S            # 4096 total rows
QF = HL * HD          # 256 local q features
POS = S // M          # 256 positions owned per core per batch

bf16 = ml_dtypes.bfloat16

_CACHE = {}
RUN_OPTS = {}          # test harness may set {"trace": True}
LAST_RESULT = [None]   # test harness reads profiling info from here


def _build_nc():
    import concourse.bacc as bacc
    import concourse.mybir as mybir
    from concourse import tile

    F32, BF16 = mybir.dt.float32, mybir.dt.bfloat16
    Exp = mybir.ActivationFunctionType.Exp

    nc = bacc.Bacc("TRN2", target_bir_lowering=False, debug=False, num_devices=M)

    xt_p = nc.declare_dram_parameter("xt", [D, RT], BF16, isOutput=False)
    cos_p = nc.declare_dram_parameter("cos2", [128, S], BF16, isOutput=False)
    ssin_p = nc.declare_dram_parameter("ssin2", [128, S], BF16, isOutput=False)
    wq_p = nc.declare_dram_parameter("wqs", [D, QF], BF16, isOutput=False)
    wkv_p = nc.declare_dram_parameter("wkvs", [D, 2 * HD], BF16, isOutput=False)
    wo_p = nc.declare_dram_parameter("woall", [D, D], BF16, isOutput=False)
    tri_p = nc.declare_dram_parameter("tri4", [128, 512], BF16, isOutput=False)
    id_p = nc.declare_dram_parameter("ident", [128, 128], BF16, isOutput=False)
    out_p = nc.declare_dram_parameter("out", [B * POS, D], BF16, isOutput=True)

    with tile.TileContext(nc) as tc:
        with tc.tile_pool(name="dram", bufs=1, space="DRAM") as dram, \
             tc.tile_pool(name="persist", bufs=1) as per, \
             tc.tile_pool(name="xload", bufs=2) as xload, \
             tc.tile_pool(name="work", bufs=2) as work, \
             tc.tile_pool(name="pwork", bufs=2) as pwork, \
             tc.tile_pool(name="ostage", bufs=2) as ostage, \
             tc.tile_pool(name="atpool", bufs=2) as atpool, \
             tc.tile_pool(name="pj", bufs=2, space="PSUM") as pj, \
             tc.tile_pool(name="sc", bufs=2, space="PSUM") as sc, \
             tc.tile_pool(name="acc", bufs=2, space="PSUM") as acc:

            # ---- resident loads -------------------------------------------------
            # rc0/rc1 x slices first: they gate the first two projection rounds
            xr0 = xload.tile([128, 16, 512], BF16, tag="x")
            for _k in range(8):
                nc.sync.dma_start(
                    out=xr0[:, 2 * _k:2 * (_k + 1), :],
                    in_=xt_p[256 * _k:256 * (_k + 1), 0:512].rearrange(
                        "(n p) f -> p n f", p=128))
            wq_sb = per.tile([128, 16, QF], BF16, tag="wq")
            for _h in range(2):
                for _d in range(4):
                    nc.sync.dma_start(
                        out=wq_sb[:, 4 * _d:4 * (_d + 1), 128 * _h:128 * (_h + 1)],
                        in_=wq_p[512 * _d:512 * (_d + 1),
                                 128 * _h:128 * (_h + 1)].rearrange("(n p) f -> p n f", p=128))
            wkv_sb = per.tile([128, 16, 2 * HD], BF16, tag="wkv")
            for _d in range(4):
                nc.sync.dma_start(
                    out=wkv_sb[:, 4 * _d:4 * (_d + 1), :],
                    in_=wkv_p[512 * _d:512 * (_d + 1), :].rearrange("(n p) f -> p n f", p=128))
            cos_sb = per.tile([128, S], BF16, tag="cos")
            for _h in range(4):
                nc.sync.dma_start(out=cos_sb[:, 512 * _h:512 * (_h + 1)],
                                  in_=cos_p[:, 512 * _h:512 * (_h + 1)])
            ssin_sb = per.tile([128, S], BF16, tag="ssin")
            for _h in range(4):
                nc.sync.dma_start(out=ssin_sb[:, 512 * _h:512 * (_h + 1)],
                                  in_=ssin_p[:, 512 * _h:512 * (_h + 1)])
            tri_sb = per.tile([128, 512], BF16, tag="tri")
            nc.sync.dma_start(out=tri_sb[:], in_=tri_p[:, :])
            id_sb = per.tile([128, 128], BF16, tag="ident")
            nc.sync.dma_start(out=id_sb[:], in_=id_p[:, :])
            xr1 = xload.tile([128, 16, 512], BF16, tag="x")
            for _k in range(8):
                nc.sync.dma_start(
                    out=xr1[:, 2 * _k:2 * (_k + 1), :],
                    in_=xt_p[256 * _k:256 * (_k + 1), 512:1024].rearrange(
                        "(n p) f -> p n f", p=128))
            # full wo, pre-transposed: wo_sb[p, d, of] = wo[of, 128*d+p]
            # (loaded late, after rc0's x, to keep startup DMA bandwidth for x)
            wo_sb = per.tile([128, 16, D], BF16, tag="wo")

            # q/kt live on all 128 partitions: rows 64-127 duplicate rows 0-63 so
            # score matmuls run as two concurrent 64-row PE tiles (T0 + T8)
            q_flat = per.tile([128, B * NCH * HL * CH], BF16, tag="qflat")  # (b,c,hh,pos)
            kt_sb = per.tile([128, RT], BF16, tag="kt")
            at_sb = [per.tile([128, RT], BF16, tag=f"at{i}", name=f"at{i}") for i in range(2)]
            vau = [per.tile([128, HD + 1], BF16, tag=f"vau{i}", name=f"vau{i}") for i in range(RT // 128)]
            atall = [None]  # shared SBUF buffer, reloaded per batch

            qv = q_flat[0:64, :].rearrange("p (b c h x) -> p b c h x", b=B, c=NCH, h=HL, x=CH)

            # ---- AllToAll buffers (per half-batch: 4 x 0.5MB) ------------------
            # half (b,h) covers batch-b positions [1024h, 1024h+1024); shard j =
            # my 256 features x 128 positions -> dest core j, which owns batch-b
            # positions 1024h + 128j .. +128.
            a2a_in = [dram.tile([M * QF, CH], BF16, tag=f"a2ai{i}", name=f"a2ai{i}")
                      for i in range(4)]
            a2a_out = [dram.tile([M * QF, CH], BF16, tag=f"a2ao{i}", name=f"a2ao{i}")
                       for i in range(4)]

            def emit_a2a_block(b, cc):
                # chunks cc-1, cc (= shards j0, j0+1 of half h) -> a2a_in rows
                h, j0 = cc // 8, (cc % 8) - 1
                for t in range(2):
                    dst = a2a_in[2 * b + h].rearrange(
                        "(j t p) c -> t p j c", t=2, p=128)[t][:, j0:j0 + 2, :]
                    src = at_sb[t][:, b * S + CH * (cc - 1):b * S + CH * (cc + 1)
                                   ].rearrange("p (j c) -> p j c", j=2)
                    nc.sync.dma_start(out=dst, in_=src)

            def emit_a2a(b, h):
                nc.gpsimd.collective_compute(
                    "AllToAll", mybir.AluOpType.bypass,
                    replica_groups=[list(range(M))],
                    ins=[a2a_in[2 * b + h].opt()], outs=[a2a_out[2 * b + h].opt()])

            def emit_atall_load(b, h):
                # a2a_out rows 128*d+p -> atall[p, d, :]
                at_t = atpool.tile([128, 16, CH], BF16, tag="atall", name="atall")
                atall[0] = at_t
                for k in range(4):
                    nc.sync.dma_start(
                        out=at_t[:, 4 * k:4 * (k + 1), :],
                        in_=a2a_out[2 * b + h][512 * k:512 * (k + 1), :].rearrange(
                            "(d p) c -> p d c", p=128))

            def emit_wo_chain(b, h, o4):
                # out.T chain: lhsT = attention block (stationary), wo streams N=512
                ps = pj.tile([128, 512], F32, tag="pj", name="wops")
                for d in range(16):
                    nc.tensor.matmul(ps[:], atall[0][:, d, :],
                                     wo_sb[:, d, 512 * o4:512 * (o4 + 1)],
                                     start=(d == 0), stop=(d == 15))
                ob = ostage.tile([128, 512], BF16, tag="ob", name="ob")
                nc.any.tensor_copy(ob[:], ps[:])
                nc.sync.dma_start(
                    out=out_p[256 * b + 128 * h:256 * b + 128 * (h + 1),
                              512 * o4:512 * (o4 + 1)],
                    in_=ob[:])

            def emit_attn(b, c):
                o_ps = acc.tile([HD + 1, 512], F32, tag="acc", name="ops")
                qc0 = (b * NCH + c) * 512
                for j0 in range(0, c + 1, 2):
                    js = [j for j in (j0, j0 + 1) if j <= c]
                    s_ps = sc.tile([128, 1024], F32, tag="sc", name="sps")
                    for idx, j in enumerate(js):
                        lo = 64 * idx  # idx 0 -> PE row-tile T0, idx 1 -> T8
                        nc.tensor.matmul(
                            s_ps[:, 512 * idx:512 * (idx + 1)],
                            kt_sb[lo:lo + 64, b * S + CH * j: b * S + CH * (j + 1)],
                            q_flat[lo:lo + 64, qc0:qc0 + 512], start=True, stop=True)
                    nw = 512 * len(js)
                    p_sb = pwork.tile([128, 1024], BF16, tag="p", name="psb")
                    nc.scalar.activation(p_sb[:, 0:nw], s_ps[:, 0:nw], Exp, scale=0.125)
                    if c in js:
                        idx = js.index(c)
                        nc.vector.tensor_mul(p_sb[:, 512 * idx:512 * (idx + 1)],
                                             p_sb[:, 512 * idx:512 * (idx + 1)], tri_sb[:])
                    for idx, j in enumerate(js):
                        nc.tensor.matmul(o_ps[:], vau[b * NCH + j][:],
                                         p_sb[:, 512 * idx:512 * (idx + 1)],
                                         start=(j == 0), stop=(j == c))
                # normalization: 1/den from the PSUM ones-row
                bc = pwork.tile([64, 512], F32, tag="bc", name="bct")
                nc.vector.tensor_copy(bc[0:1, :], o_ps[HD:HD + 1, :])
                rrow = pwork.tile([1, 512], F32, tag="rrow", name="rrow")
                nc.vector.reciprocal_approx_fast(rrow[:], bc[0:1, :])
                nc.gpsimd.partition_broadcast(bc[:], rrow[:])
                for hh in range(HL):
                    nc.vector.tensor_mul(
                        at_sb[hh // 2][64 * (hh % 2):64 * (hh % 2) + 64,
                                       b * S + CH * c: b * S + CH * (c + 1)],
                        o_ps[0:64, 128 * hh:128 * (hh + 1)],
                        bc[:, 128 * hh:128 * (hh + 1)])

            # ---- projections + RoPE, per 512-row slice -------------------------
            pending_blocks = []

            for rc in range(8):
                b, cg = rc // 4, rc % 4
                if rc == 0:
                    xr = xr0
                elif rc == 1:
                    xr = xr1
                else:
                    xr = xload.tile([128, 16, 512], BF16, tag="x")
                    for _k in range(8):
                        nc.sync.dma_start(
                            out=xr[:, 2 * _k:2 * (_k + 1), :],
                            in_=xt_p[256 * _k:256 * (_k + 1),
                                     512 * rc:512 * (rc + 1)].rearrange(
                                         "(n p) f -> p n f", p=128))
                # flush the previous slice's a2a shard writes now — after the x
                # triggers, so their norm-waits can't block the x pipeline
                for (pb, pcc) in pending_blocks:
                    emit_a2a_block(pb, pcc)
                pending_blocks.clear()
                if rc == 2:
                    emit_a2a(0, 0)
                if rc == 1:
                    for _k in range(8):
                        nc.sync.dma_start(
                            out=wo_sb[:, 2 * _k:2 * (_k + 1), :],
                            in_=wo_p[256 * _k:256 * (_k + 1), :].rearrange(
                                "(d p) f -> p d f", p=128))
                if rc == 4:
                    emit_atall_load(0, 0)
                if rc == 6:
                    emit_atall_load(0, 1)
                if rc == 7:
                    emit_atall_load(1, 0)
                cs = cos_sb[:, 512 * cg:512 * (cg + 1)]
                sn = ssin_sb[:, 512 * cg:512 * (cg + 1)]

                # Q: two 128-feature chunks (2 heads each)
                for f in range(2):
                    ps = pj.tile([128, 512], F32, tag="pj")
                    for d in range(16):
                        nc.tensor.matmul(ps[:], wq_sb[:, d, 128 * f:128 * (f + 1)],
                                         xr[:, d, :], start=(d == 0), stop=(d == 15))
                    t1 = work.tile([128, 512], BF16, tag="t1")
                    nc.vector.tensor_mul(t1[:], ps[:], cs)
                    sw = work.tile([128, 512], BF16, tag="sw")
                    for a, bq in ((0, 1), (1, 0), (2, 3), (3, 2)):
                        nc.scalar.copy(sw[32 * a:32 * (a + 1), :], ps[32 * bq:32 * (bq + 1), :])
                    t2 = work.tile([128, 512], BF16, tag="t2")
                    nc.vector.tensor_mul(t2[:], sw[:], sn)
                    for hf in range(2):
                        hh = 2 * f + hf
                        dst = qv[:, b, 4 * cg:4 * (cg + 1), hh, :]
                        nc.vector.tensor_add(
                            dst,
                            t1[64 * hf:64 * (hf + 1), :].rearrange("p (a x) -> p a x", x=CH),
                            t2[64 * hf:64 * (hf + 1), :].rearrange("p (a x) -> p a x", x=CH))
                    qv2 = q_flat[64:128, :].rearrange("p (b c h x) -> p b c h x",
                                                      b=B, c=NCH, h=HL, x=CH)
                    nc.scalar.copy(qv2[:, b, 4 * cg:4 * (cg + 1), 2 * f:2 * f + 2, :],
                                   qv[:, b, 4 * cg:4 * (cg + 1), 2 * f:2 * f + 2, :])

                # K+V packed: one full-array matmul chain (k rows 0-63, v rows 64-127)
                ps = pj.tile([128, 512], F32, tag="pj")
                for d in range(16):
                    nc.tensor.matmul(ps[:], wkv_sb[:, d, :], xr[:, d, :],
                                     start=(d == 0), stop=(d == 15))
                t1 = work.tile([128, 512], BF16, tag="t1")
                nc.vector.tensor_mul(t1[0:64, :], ps[0:64, :], cs[0:64, :])
                sw = work.tile([128, 512], BF16, tag="sw")
                nc.scalar.copy(sw[0:32, :], ps[32:64, :])
                nc.scalar.copy(sw[32:64, :], ps[0:32, :])
                t2 = work.tile([128, 512], BF16, tag="t2")
                nc.vector.tensor_mul(t2[0:64, :], sw[0:64, :], sn[0:64, :])
                nc.vector.tensor_add(kt_sb[0:64, 512 * rc:512 * (rc + 1)], t1[0:64, :], t2[0:64, :])
                nc.scalar.copy(kt_sb[64:128, 512 * rc:512 * (rc + 1)],
                               kt_sb[0:64, 512 * rc:512 * (rc + 1)])

                vt = work.tile([128, 512], BF16, tag="sw")
                nc.vector.tensor_copy(vt[0:64, :], ps[64:128, :])
                for t in range(4):
                    tp = acc.tile([128, 64], F32, tag="acc")
                    nc.tensor.matmul(tp[:], vt[0:64, 128 * t:128 * (t + 1)], id_sb[0:64, 0:64],
                                     start=True, stop=True)
                    vtile = vau[4 * rc + t]
                    nc.vector.tensor_copy(vtile[:, 0:HD], tp[:])
                    nc.vector.memset(vtile[:, HD:HD + 1], 1.0)

                for cc in range(4 * cg, 4 * cg + 4):
                    emit_attn(b, cc)
                    # queue finished 2-chunk shard pairs for the A2A inputs
                    if cc % 2 == 1:
                        pending_blocks.append((b, cc))
                    # trigger each half's A2A once its shards are written (the
                    # cross-batch ones a little late so the gpsimd queue isn't
                    # head-of-line blocked on the a2a_in DMA waits)
                    if b == 1 and cc == 1:
                        emit_a2a(0, 1)
                    if b == 1 and cc == 9:
                        emit_a2a(1, 0)
                    # interleave wo chains of already-exchanged halves
                    if b == 1 and 4 <= cc < 8:
                        emit_wo_chain(0, 0, cc - 4)
                    if b == 1 and 8 <= cc < 12:
                        emit_wo_chain(0, 1, cc - 8)

            for (pb, pcc) in pending_blocks:
                emit_a2a_block(pb, pcc)
            pending_blocks.clear()
            emit_a2a(1, 1)
            # wo chains for b1h0 fill the PE during the final AllToAll
            for o4 in range(4):
                emit_wo_chain(1, 0, o4)
            emit_atall_load(1, 1)
            for o4 in range(4):
                emit_wo_chain(1, 1, o4)

    nc.compile()
    return nc


def _stage(x, cos, sin, wq, wk, wv, wo):
    xt = np.ascontiguousarray(x.reshape(RT, D).T).astype(bf16)
    cosT = cos.T.astype(np.float32)                      # [64, S]
    sinT = sin.T.astype(np.float32)
    cos2 = np.concatenate([cosT, cosT], axis=0).astype(bf16)       # [128, S]
    ssin1 = np.concatenate([-sinT[:HD // 2], sinT[HD // 2:]], axis=0)
    ssin2 = np.concatenate([ssin1, ssin1], axis=0).astype(bf16)    # [128, S]
    tri4 = np.tile(np.triu(np.ones((CH, CH), np.float32)), (1, 4)).astype(bf16)
    ident = np.eye(128, dtype=np.float32).astype(bf16)
    woall = np.ascontiguousarray(wo.T).astype(bf16)      # [af, of]

    in_maps = []
    for m in range(M):
        in_maps.append({
            "xt": xt,
            "cos2": cos2,
            "ssin2": ssin2,
            "wqs": np.ascontiguousarray(wq[QF * m:QF * (m + 1), :].T).astype(bf16),
            "wkvs": np.ascontiguousarray(np.concatenate(
                [wk[HD * m:HD * (m + 1), :].T, wv[HD * m:HD * (m + 1), :].T], axis=1)).astype(bf16),
            "woall": woall,
            "tri4": tri4,
            "ident": ident,
        })
    return in_maps


def kernel(x, cos, sin, wq, wk, wv, wo):
    from concourse.bass_utils import run_bass_kernel_spmd

    if "nc" not in _CACHE:
        _CACHE["nc"] = _build_nc()
    nc = _CACHE["nc"]

    in_maps = _stage(x, cos, sin, wq, wk, wv, wo)
    res = run_bass_kernel_spmd(nc, in_maps, list(range(M)), **RUN_OPTS)
    LAST_RESULT[0] = res

    full = np.empty((B, S, D), np.float32)
    for m in range(M):
        o = np.asarray(res.results[m]["out"]).astype(np.float32)   # [4*CH, D]
        for b in range(B):
            for h in range(2):
                full[b, 1024 * h + CH * m:1024 * h + CH * (m + 1), :] = \
                    o[256 * b + CH * h:256 * b + CH * (h + 1), :]
    return full
